# revision 1
# baseline (speedup 1.0000x reference)
"""SRP layer distributed Bass kernel for TRN2 (v6).

Math (full problem): out = Psi_c @ x.T @ x with Psi_c = Psi - rowmean(Psi).
  x [D, N] f32, Psi [O, N] f32, out [O, N] f32  (D=4096, N=8192, O=2048)

Distribution over 8 cores as a 2x4 grid: core c -> (i = c % 2: n-half,
j = c // 2: o-quarter). Per core:
  x_i  [D, NL]    (NL = N/2)
  psi_ji [OL, NL] (OL = O/4)
  rs   [OT, 128]  host-precomputed psi_ji.sum(axis=1), o-tile-major rows
  xrs  [1, D]     host-precomputed x_i.sum(axis=1)
  out_ji [OL, NL]

Key structure (TensorE-facing data bf16 via SWDGE cast-DMA, PSUM f32):
  - mm1 runs on UNCENTERED psi; centering is a rank-1 correction
    tmp -= mean[o] * xrs_local[d] applied as K=1 matmuls into the same
    PSUM accumulation group (mean from a tiny pair-AllReduce of rs that
    runs off the critical path).
  - mm1 by d-chunks of 512: x rows cast-loaded bf16, PE-transposed into
    xT; next chunk's transposes interleave between mm groups.
  - tmp halves pair-AllReduced in bf16, overlapped with mm1 tail and
    mm2 pass A (kd lower half).
  - mm2 streams the natural-layout bf16 x copy written during mm1.
"""

from contextlib import ExitStack

import concourse.bacc as bacc
import concourse.mybir as mybir
import concourse.tile as tile
from concourse.masks import make_identity

F32 = mybir.dt.float32
BF = mybir.dt.bfloat16


def build_srp_kernel(
    D=4096,
    NL=4096,
    OL=512,
    NTOT=8192,
    n_cores=8,
    groups=((0, 1), (2, 3), (4, 5), (6, 7)),
    ar_dtype=BF,
):
    OT = OL // 128      # o-tiles
    KN = NL // 128      # n-tiles (mm1 contraction)
    DC = D // 512       # d-chunks (mm1 output cols)
    ND = NL // 512      # n-chunks (mm2 output cols)
    KD = D // 128       # d-tiles (mm2 contraction)
    DH = D // 2         # half of d (AR chunk)
    assert DC % 2 == 0 and KD % 2 == 0

    groups = [list(g) for g in groups]

    nc = bacc.Bacc("TRN2", target_bir_lowering=False, debug=False,
                   num_devices=n_cores)
    x_ext = nc.dram_tensor("x", [D, NL], F32, kind="ExternalInput")
    psi_ext = nc.dram_tensor("psi", [OL, NL], F32, kind="ExternalInput")
    rs_ext = nc.dram_tensor("rs", [1, OL], F32, kind="ExternalInput")
    xrs_ext = nc.dram_tensor("xrs", [1, D], F32, kind="ExternalInput")
    out_ext = nc.dram_tensor("out", [OL, NL], F32, kind="ExternalOutput")

    with ExitStack() as stack:
        tc = stack.enter_context(tile.TileContext(nc))
        dram = stack.enter_context(tc.tile_pool(name="dram", bufs=1, space="DRAM"))
        const = stack.enter_context(tc.tile_pool(name="const", bufs=1))
        ps = stack.enter_context(tc.tile_pool(name="ps", bufs=1, space="PSUM"))

        ident = const.tile([128, 128], BF, tag="ident", bufs=1)
        make_identity(nc, ident[:])

        rs_in = dram.tile([1, OL], F32, tag="rs_in", bufs=1)
        rs_out = dram.tile([1, OL], F32, tag="rs_out", bufs=1)
        tmp_in = [dram.tile([OL, DH], ar_dtype, tag=f"tmp_in{h}", bufs=1,
                            name=f"tmp_in{h}")
                  for h in range(2)]
        tmp_out = [dram.tile([OL, DH], ar_dtype, tag=f"tmp_out{h}", bufs=1,
                             name=f"tmp_out{h}")
                   for h in range(2)]
        x_bf_dram = dram.tile([D, NL], BF, tag="x_bf_dram", bufs=1)

        # mean_neg_row[t, :] = -rowmean(Psi) for o-tile t (bf16)
        mean_neg_row = const.tile([1, OL], BF, tag="mean_neg_row", bufs=1)
        xrs_bf = const.tile([1, D], BF, tag="xrs_bf", bufs=1)

        # ============ phase A + mm1 scope ============
        with tc.tile_pool(name="sb1", bufs=1) as sb:
            x_bf = {}

            def x_chunk_load(dc):
                for dt in range(4):
                    xb = sb.tile([128, NL], BF, tag="x_bf", bufs=6,
                                 name=f"x_bf{dc}_{dt}")
                    x_bf[(dc, dt)] = xb
                    row = dc * 512 + dt * 128
                    nc.gpsimd.dma_start(xb[:], x_ext[row: row + 128, :])
                    # natural-layout bf16 copy for mm2 streaming
                    nc.scalar.dma_start(x_bf_dram[row: row + 128, :], xb[:])

            def x_chunk_transpose(dc, xT, k_lo, k_hi):
                for k in range(k_lo, k_hi):
                    pt = ps.tile([128, 512], BF, tag="pst", bufs=2,
                                 name=f"pstx{dc}_{k}")
                    for dt in range(4):
                        nc.tensor.transpose(
                            pt[:, dt * 128:(dt + 1) * 128],
                            x_bf[(dc, dt)][:, k * 128:(k + 1) * 128],
                            ident[:])
                    nc.vector.tensor_copy(xT[:, k * 512:(k + 1) * 512], pt[:])

            x_chunk_load(0)
            nc.scalar.dma_start(rs_in[:], rs_ext[:])
            nc.gpsimd.collective_compute(
                "AllReduce", mybir.AluOpType.add, replica_groups=groups,
                ins=[rs_in.opt()], outs=[rs_out.opt()])
            psi_bf = []
            for t in range(OT):
                pb = sb.tile([128, NL], BF, tag="psi_bf", bufs=OT,
                             name=f"psi_bf{t}")
                psi_bf.append(pb)
                nc.gpsimd.dma_start(pb[:], psi_ext[t * 128:(t + 1) * 128, :])
            nc.gpsimd.dma_start(xrs_bf[:], xrs_ext[:])
            mn_f = sb.tile([1, OL], F32, tag="mn_f", bufs=1)
            nc.scalar.dma_start(mn_f[:], rs_out[:])
            nc.vector.tensor_scalar_mul(mn_f[:], mn_f[:], -1.0 / NTOT)
            nc.vector.tensor_copy(mean_neg_row[:], mn_f[:])

            # psiT: block k at cols [k*OL, (k+1)*OL) = [128(n), OL(o)] bf16
            psiT = sb.tile([128, KN * OL], BF, tag="psiT", bufs=1)
            xT_bufs = [sb.tile([128, KN * 512], BF, tag="xT", bufs=2,
                               name=f"xT{b}")
                       for b in range(2)]

            # chunk-0 transposes + uncentered psi transposes (no AR dep)
            x_chunk_load(1)
            x_chunk_transpose(0, xT_bufs[0], 0, KN)
            for k in range(KN):
                pt = ps.tile([128, OL], BF, tag="pst", bufs=2, name=f"pstp{k}")
                for t in range(OT):
                    nc.tensor.transpose(pt[:, t * 128:(t + 1) * 128],
                                        psi_bf[t][:, k * 128:(k + 1) * 128],
                                        ident[:])
                nc.vector.tensor_copy(psiT[:, k * OL:(k + 1) * OL], pt[:])

            # ---- mm1 ----
            for dc in range(DC):
                xT = xT_bufs[dc % 2]
                if dc + 2 < DC:
                    x_chunk_load(dc + 2)
                mm = [ps.tile([128, 512], F32, tag="mmps", bufs=6,
                              name=f"mm1_{dc}_{_ot}")
                      for _ot in range(OT)]
                for ot in range(OT):
                    for k in range(KN):
                        nc.tensor.matmul(
                            mm[ot][:],
                            psiT[:, k * OL + ot * 128: k * OL + (ot + 1) * 128],
                            xT[:, k * 512:(k + 1) * 512],
                            start=(k == 0), stop=False)
                    # rank-1 centering correction: tmp -= mean[o] * xrs[d]
                    for q in range(4):
                        nc.tensor.matmul(
                            mm[ot][:, q * 128:(q + 1) * 128],
                            mean_neg_row[0:1, ot * 128:(ot + 1) * 128],
                            xrs_bf[0:1, dc * 512 + q * 128:
                                   dc * 512 + (q + 1) * 128],
                            start=False, stop=(q == 3))
                    # interleave next chunk's transposes between mm groups
                    if dc + 1 < DC:
                        x_chunk_transpose(dc + 1, xT_bufs[(dc + 1) % 2],
                                          ot * (KN // OT),
                                          (ot + 1) * (KN // OT))
                h, dci = dc // (DC // 2), dc % (DC // 2)
                for ot in range(OT):
                    stage = sb.tile([128, 512], ar_dtype, tag="t1stage", bufs=8,
                                    name=f"t1stage{dc}_{ot}")
                    nc.vector.tensor_copy(stage[:], mm[ot][:])
                    nc.scalar.dma_start(
                        tmp_in[h][ot * 128:(ot + 1) * 128,
                                  dci * 512:(dci + 1) * 512],
                        stage[:])
                if dc == DC // 2 - 1:
                    nc.gpsimd.collective_compute(
                        "AllReduce", mybir.AluOpType.add, replica_groups=groups,
                        ins=[tmp_in[0].opt()], outs=[tmp_out[0].opt()])
            nc.gpsimd.collective_compute(
                "AllReduce", mybir.AluOpType.add, replica_groups=groups,
                ins=[tmp_in[1].opt()], outs=[tmp_out[1].opt()])

        # ============ mm2 scope ============
        with tc.tile_pool(name="sb2", bufs=1) as sb:
            tmpT = sb.tile([128, KD * OL], BF, tag="tmpT", bufs=1)
            out_part = [sb.tile([128, NL], F32, tag=f"out_part{ot}", bufs=1,
                                name=f"out_part{ot}")
                        for ot in range(OT)]
            for h in range(2):
                tmp_sb = []
                for t in range(OT):
                    tl = sb.tile([128, DH], ar_dtype, tag="tmp_sb", bufs=OT,
                                 name=f"tmp_sb{h}_{t}")
                    nc.scalar.dma_start(tl[:], tmp_out[h][t * 128:(t + 1) * 128, :])
                    tmp_sb.append(tl)
                for kdl in range(KD // 2):
                    kd = h * (KD // 2) + kdl
                    pt = ps.tile([128, OL], BF, tag="pst", bufs=2,
                                 name=f"pst2_{kd}")
                    for t in range(OT):
                        nc.tensor.transpose(pt[:, t * 128:(t + 1) * 128],
                                            tmp_sb[t][:, kdl * 128:(kdl + 1) * 128],
                                            ident[:])
                    nc.vector.tensor_copy(tmpT[:, kd * OL:(kd + 1) * OL], pt[:])
                for ncn in range(ND):
                    mm = [ps.tile([128, 512], F32, tag="mmps", bufs=6,
                                  name=f"mm2_{h}_{ncn}_{_ot}")
                          for _ot in range(OT)]
                    for kdl in range(KD // 2):
                        kd = h * (KD // 2) + kdl
                        x2b = sb.tile([128, 512], BF, tag="x2b", bufs=8,
                                      name=f"x2b{h}_{ncn}_{kdl}")
                        dma_eng = nc.sync if (kdl % 2 == 0) else nc.scalar
                        dma_eng.dma_start(
                            x2b[:], x_bf_dram[kd * 128:(kd + 1) * 128,
                                              ncn * 512:(ncn + 1) * 512])
                        for ot in range(OT):
                            nc.tensor.matmul(
                                mm[ot][:],
                                tmpT[:, kd * OL + ot * 128: kd * OL + (ot + 1) * 128],
                                x2b[:],
                                start=(kdl == 0), stop=(kdl == KD // 2 - 1))
                    for ot in range(OT):
                        if h == 0:
                            nc.vector.tensor_copy(
                                out_part[ot][:, ncn * 512:(ncn + 1) * 512],
                                mm[ot][:])
                        else:
                            ostage = sb.tile([128, 512], F32, tag="ostage",
                                             bufs=8, name=f"ostage{ncn}_{ot}")
                            nc.vector.tensor_tensor(
                                ostage[:], mm[ot][:],
                                out_part[ot][:, ncn * 512:(ncn + 1) * 512],
                                op=mybir.AluOpType.add)
                            nc.scalar.dma_start(
                                out_ext[ot * 128:(ot + 1) * 128,
                                        ncn * 512:(ncn + 1) * 512],
                                ostage[:])
    nc.compile()
    return nc


def make_in_maps(x, Psi, n_cores=8, NL=4096, OL=512):
    """Shard full inputs for the 2x4 grid, with host-side row-sum stats."""
    import numpy as np
    OT = OL // 128
    in_maps = []
    for c in range(n_cores):
        i, j = c % 2, c // 2
        xs = np.ascontiguousarray(x[:, i * NL:(i + 1) * NL])
        ps_ = np.ascontiguousarray(Psi[j * OL:(j + 1) * OL, i * NL:(i + 1) * NL])
        in_maps.append({
            "x": xs,
            "psi": ps_,
            "rs": ps_.sum(axis=1, dtype=np.float64).astype(np.float32).reshape(1, -1),
            "xrs": xs.sum(axis=1, dtype=np.float64).astype(np.float32).reshape(1, -1),
        })
    return in_maps


# ---------------- harness-facing wrapper ----------------
import numpy as np

_NC_CACHE = {}

D_FULL, N_FULL, O_FULL = 4096, 8192, 2048
NL_, OL_ = 4096, 512
N_CORES = 8
GROUPS = ((0, 1), (2, 3), (4, 5), (6, 7))


def _get_nc():
    if "nc" not in _NC_CACHE:
        _NC_CACHE["nc"] = build_srp_kernel(
            D=D_FULL, NL=NL_, OL=OL_, NTOT=N_FULL,
            n_cores=N_CORES, groups=GROUPS)
    return _NC_CACHE["nc"]


def kernel(x, Psi):
    """out = (Psi - rowmean(Psi)) @ x.T @ x on 8 TRN2 NeuronCores."""
    from concourse.bass_utils import run_bass_kernel_spmd
    x = np.asarray(x, dtype=np.float32)
    Psi = np.asarray(Psi, dtype=np.float32)
    assert x.shape == (D_FULL, N_FULL) and Psi.shape == (O_FULL, N_FULL)
    nc = _get_nc()
    in_maps = make_in_maps(x, Psi, n_cores=N_CORES, NL=NL_, OL=OL_)
    res = run_bass_kernel_spmd(nc, in_maps, core_ids=list(range(N_CORES)))
    out = np.empty((O_FULL, N_FULL), dtype=np.float32)
    for c in range(N_CORES):
        i, j = c % 2, c // 2
        out[j * OL_:(j + 1) * OL_, i * NL_:(i + 1) * NL_] = res.results[c]["out"]
    return out



# revision 2
# speedup vs baseline: 1.4996x; 1.4996x over previous
"""SRP layer distributed Bass kernel for TRN2 (v7).

Math (full problem): out = Psi_c @ x.T @ x with Psi_c = Psi - rowmean(Psi).
  x [D, N] f32, Psi [O, N] f32, out [O, N] f32  (D=4096, N=8192, O=2048)

Distribution over 8 cores as a 2x4 grid: core c -> (i = c % 2: n-half,
j = c // 2: o-quarter). All heavy layout work happens on the HOST:
Psi is centered (global row mean) and transposed, x is transposed, and
both x layouts are cast to bf16 and pre-tiled so that every device load
is a contiguous [128, 4096] (1 MB) DMA. The device then does NOTHING but
matmuls: no PE transposes, no centering correction.

Per core:
  xt   [4096, 4096] bf16: xT tiled - block b = p*4+g holds rows of
       x_i^T for k-tiles 8g..8g+7 (n), d-panel p (512 cols)
  xn   [4096, 4096] bf16: x natural tiled - block b = ncn*4+g holds
       kd-tiles 8g..8g+7 (d), n-chunk ncn (512 cols)
  psit [512, 4096] bf16: Psi_c^T tiled - block g holds k-tiles 8g..8g+7
       (n) by all 512 o columns
  out  [512, 4096] f32 natural [o, n-half]

Pipeline: mm1 computes tmpT[d, o] = sum_n x[d,n] psi_c[o,n] panel by
panel (stationary = xT blocks, moving = psiT); each finished 512-row
panel is pair-AllReduced immediately (8 x 512 KB chunks, overlapped with
later panels). mm2 (stationary = tmpT blocks from the AR, moving = x
natural) streams n-chunks and writes out f32 directly.
"""

from contextlib import ExitStack

import concourse.bacc as bacc
import concourse.mybir as mybir
import concourse.tile as tile

F32 = mybir.dt.float32
BF = mybir.dt.bfloat16

D = 4096          # d_feat
NL = 4096         # local n (N/2)
OL = 512          # local o (O/4)
NP = 8            # d panels (mm1 output chunks / AR chunks)
NCN = 8           # n chunks (mm2 output chunks)
NG = 4            # k-groups of 8 tiles per 4096-wide sbuf tile
GROUPS = [[0, 1], [2, 3], [4, 5], [6, 7]]


def build_srp_kernel(n_cores=8, groups=GROUPS):
    nc = bacc.Bacc("TRN2", target_bir_lowering=False, debug=False,
                   num_devices=n_cores)
    xt_ext = nc.dram_tensor("xt", [D, 4096], BF, kind="ExternalInput")
    xn_ext = nc.dram_tensor("xn", [D, 4096], BF, kind="ExternalInput")
    psit_ext = nc.dram_tensor("psit", [OL, 4096], BF, kind="ExternalInput")
    out_ext = nc.dram_tensor("out", [OL, NL], F32, kind="ExternalOutput")

    with ExitStack() as stack:
        tc = stack.enter_context(tile.TileContext(nc))
        dram = stack.enter_context(tc.tile_pool(name="dram", bufs=1, space="DRAM"))
        ps = stack.enter_context(tc.tile_pool(name="ps", bufs=1, space="PSUM"))
        sb = stack.enter_context(tc.tile_pool(name="sb", bufs=1))

        tmp_in = [dram.tile([OL, OL], BF, tag=f"tmp_in{p}", bufs=1,
                            name=f"tmp_in{p}")
                  for p in range(NP)]
        tmp_out = [dram.tile([OL, OL], BF, tag=f"tmp_out{p}", bufs=1,
                             name=f"tmp_out{p}")
                   for p in range(NP)]

        # ---- resident psiT (moving operand of mm1) ----
        psi_sb = []
        for g in range(NG):
            pg = sb.tile([128, 4096], BF, tag="psi", bufs=NG, name=f"psi{g}")
            nc.scalar.dma_start(pg[:], psit_ext[g * 128:(g + 1) * 128, :])
            psi_sb.append(pg)

        # ---- xn chunk 0 early on scalar (needed right at mm1->mm2 edge) ----
        xn_tiles = {}

        def xn_load(ncn, eng):
            for g in range(NG):
                t = sb.tile([128, 4096], BF, tag="xn", bufs=6,
                            name=f"xn{ncn}_{g}")
                eng.dma_start(t[:], xn_ext[(ncn * NG + g) * 128:
                                           (ncn * NG + g + 1) * 128, :])
                xn_tiles[(ncn, g)] = t

        xn_load(0, nc.scalar)

        # ---- mm1: tmpT[d, o] panel by panel ----
        xt_tiles = {}

        def xt_load(p):
            for g in range(NG):
                t = sb.tile([128, 4096], BF, tag="xt", bufs=6,
                            name=f"xt{p}_{g}")
                nc.sync.dma_start(t[:], xt_ext[(p * NG + g) * 128:
                                               (p * NG + g + 1) * 128, :])
                xt_tiles[(p, g)] = t

        xt_load(0)
        xt_load(1)
        for p in range(NP):
            if p + 2 < NP:
                xt_load(p + 2)
            for dtl in range(4):
                pt = ps.tile([128, 512], F32, tag="ps", bufs=8,
                             name=f"mm1_{p}_{dtl}")
                for g in range(NG):
                    xt = xt_tiles[(p, g)]
                    for kk in range(8):
                        k = 8 * g + kk
                        nc.tensor.matmul(
                            pt[:],
                            xt[:, kk * 512 + dtl * 128:
                               kk * 512 + (dtl + 1) * 128],
                            psi_sb[g][:, kk * 512:(kk + 1) * 512],
                            start=(k == 0), stop=(k == 31))
                st = sb.tile([128, 512], BF, tag="st", bufs=4,
                             name=f"st{p}_{dtl}")
                nc.vector.tensor_copy(st[:], pt[:])
                nc.scalar.dma_start(
                    tmp_in[p][dtl * 128:(dtl + 1) * 128, :], st[:])
            nc.gpsimd.collective_compute(
                "AllReduce", mybir.AluOpType.add, replica_groups=groups,
                ins=[tmp_in[p].opt()], outs=[tmp_out[p].opt()])

        # ---- mm2 stationary tiles (tmpT summed) on sync after xt ----
        ts_tiles = []
        for p in range(NP):
            for j in range(4):
                t = sb.tile([128, 512], BF, tag="ts", bufs=32,
                            name=f"ts{p}_{j}")
                nc.sync.dma_start(t[:], tmp_out[p][j * 128:(j + 1) * 128, :])
                ts_tiles.append(t)

        # ---- remaining xn loads on gpsimd (after all AR triggers) ----
        for ncn in range(1, NCN):
            xn_load(ncn, nc.gpsimd)

        # ---- mm2: out[o, n] = tmpT^T @ x ----
        for ncn in range(NCN):
            mm = [ps.tile([128, 512], F32, tag="ps", bufs=8,
                          name=f"mm2_{ncn}_{ot}")
                  for ot in range(4)]
            for g in range(NG):
                xnt = xn_tiles[(ncn, g)]
                for kk in range(8):
                    kd = 8 * g + kk
                    for ot in range(4):
                        nc.tensor.matmul(
                            mm[ot][:],
                            ts_tiles[kd][:, ot * 128:(ot + 1) * 128],
                            xnt[:, kk * 512:(kk + 1) * 512],
                            start=(kd == 0), stop=(kd == 31))
            for ot in range(4):
                os_ = sb.tile([128, 512], F32, tag="os", bufs=8,
                              name=f"os{ncn}_{ot}")
                nc.vector.tensor_copy(os_[:], mm[ot][:])
                nc.scalar.dma_start(
                    out_ext[ot * 128:(ot + 1) * 128,
                            ncn * 512:(ncn + 1) * 512], os_[:])
    nc.compile()
    return nc


# ---------------- host-side sharding / tiling ----------------
import numpy as np
import ml_dtypes

BF_NP = ml_dtypes.bfloat16

D_FULL, N_FULL, O_FULL = 4096, 8192, 2048
N_CORES = 8


def _tile_k_major(a_bf):
    """[4096 rows, C cols] -> blocks of [128, 8*C'] with k-tiles grouped 8
    per block: in[(8g+kk)*128 + r, c] -> out[(b, r, kk*C512 + c)] per 512-col
    chunk. Works for both xT (chunk axis = d panels) and x natural (chunk
    axis = n chunks).
    Input must be [4096, 4096]. Output [32*128, 4096]."""
    A5 = a_bf.reshape(4, 8, 128, 8, 512)         # g, kk, r, chunk, c
    B = A5.transpose(3, 0, 2, 1, 4)              # chunk, g, r, kk, c
    return np.ascontiguousarray(B).reshape(4096, 4096)


def _tile_psit(psit_bf):
    """[4096, 512] -> [512, 4096]: block g = [128 r, 8 kk * 512 oc]."""
    P4 = psit_bf.reshape(4, 8, 128, 512)         # g, kk, r, oc
    Q = P4.transpose(0, 2, 1, 3)                 # g, r, kk, oc
    return np.ascontiguousarray(Q).reshape(512, 4096)


def make_in_maps(x, Psi, n_cores=8):
    psi_c = (Psi.astype(np.float64)
             - Psi.astype(np.float64).mean(axis=1, keepdims=True))
    psi_c = psi_c.astype(np.float32)
    in_maps = []
    for c in range(n_cores):
        i, j = c % 2, c // 2
        xs = x[:, i * NL:(i + 1) * NL].astype(BF_NP)          # [D, NL]
        xT = np.ascontiguousarray(xs.T)                        # [NL, D]
        ps_ = psi_c[j * OL:(j + 1) * OL, i * NL:(i + 1) * NL].astype(BF_NP)
        psT = np.ascontiguousarray(ps_.T)                      # [NL, OL]
        in_maps.append({
            "xt": _tile_k_major(xT),
            "xn": _tile_k_major(xs),
            "psit": _tile_psit(psT),
        })
    return in_maps


# ---------------- harness-facing wrapper ----------------
_NC_CACHE = {}


def _get_nc():
    if "nc" not in _NC_CACHE:
        _NC_CACHE["nc"] = build_srp_kernel(n_cores=N_CORES, groups=GROUPS)
    return _NC_CACHE["nc"]


def kernel(x, Psi):
    """out = (Psi - rowmean(Psi)) @ x.T @ x on 8 TRN2 NeuronCores."""
    from concourse.bass_utils import run_bass_kernel_spmd
    x = np.asarray(x, dtype=np.float32)
    Psi = np.asarray(Psi, dtype=np.float32)
    assert x.shape == (D_FULL, N_FULL) and Psi.shape == (O_FULL, N_FULL)
    nc = _get_nc()
    in_maps = make_in_maps(x, Psi, n_cores=N_CORES)
    res = run_bass_kernel_spmd(nc, in_maps, core_ids=list(range(N_CORES)))
    out = np.empty((O_FULL, N_FULL), dtype=np.float32)
    for c in range(N_CORES):
        i, j = c % 2, c // 2
        out[j * OL:(j + 1) * OL, i * NL:(i + 1) * NL] = res.results[c]["out"]
    return out


# revision 11
# speedup vs baseline: 1.5005x; 1.0006x over previous
"""SRP layer distributed Bass kernel for TRN2 (v7).

Math (full problem): out = Psi_c @ x.T @ x with Psi_c = Psi - rowmean(Psi).
  x [D, N] f32, Psi [O, N] f32, out [O, N] f32  (D=4096, N=8192, O=2048)

Distribution over 8 cores as a 2x4 grid: core c -> (i = c % 2: n-half,
j = c // 2: o-quarter). All heavy layout work happens on the HOST:
Psi is centered (global row mean) and transposed, x is transposed, and
both x layouts are cast to bf16 and pre-tiled so that every device load
is a contiguous [128, 4096] (1 MB) DMA. The device then does NOTHING but
matmuls: no PE transposes, no centering correction.

Per core:
  xt   [4096, 4096] bf16: xT tiled - block b = p*4+g holds rows of
       x_i^T for k-tiles 8g..8g+7 (n), d-panel p (512 cols)
  xn   [4096, 4096] bf16: x natural tiled - block b = ncn*4+g holds
       kd-tiles 8g..8g+7 (d), n-chunk ncn (512 cols)
  psit [512, 4096] bf16: Psi_c^T tiled - block g holds k-tiles 8g..8g+7
       (n) by all 512 o columns
  out  [512, 4096] f32 natural [o, n-half]

Pipeline: mm1 computes tmpT[d, o] = sum_n x[d,n] psi_c[o,n] panel by
panel (stationary = xT blocks, moving = psiT); each finished 512-row
panel is pair-AllReduced immediately (8 x 512 KB chunks, overlapped with
later panels). mm2 (stationary = tmpT blocks from the AR, moving = x
natural) streams n-chunks and writes out f32 directly.
"""

from contextlib import ExitStack

import concourse.bacc as bacc
import concourse.mybir as mybir
import concourse.tile as tile

F32 = mybir.dt.float32
BF = mybir.dt.bfloat16

D = 4096          # d_feat
NL = 4096         # local n (N/2)
OL = 512          # local o (O/4)
NP = 8            # d panels (mm1 output chunks / AR chunks)
NCN = 8           # n chunks (mm2 output chunks)
NG = 4            # k-groups of 8 tiles per 4096-wide sbuf tile
GROUPS = [[0, 1], [2, 3], [4, 5], [6, 7]]


def build_srp_kernel(n_cores=8, groups=GROUPS):
    nc = bacc.Bacc("TRN2", target_bir_lowering=False, debug=False,
                   num_devices=n_cores)
    xt_ext = nc.dram_tensor("xt", [D, 4096], BF, kind="ExternalInput")
    xn_ext = nc.dram_tensor("xn", [D, 4096], BF, kind="ExternalInput")
    psit_ext = nc.dram_tensor("psit", [OL, 4096], BF, kind="ExternalInput")
    out_ext = nc.dram_tensor("out", [OL, NL], F32, kind="ExternalOutput")

    with ExitStack() as stack:
        tc = stack.enter_context(tile.TileContext(nc))
        dram = stack.enter_context(tc.tile_pool(name="dram", bufs=1, space="DRAM"))
        ps = stack.enter_context(tc.tile_pool(name="ps", bufs=1, space="PSUM"))
        sb = stack.enter_context(tc.tile_pool(name="sb", bufs=1))

        tmp_in = [dram.tile([OL, OL], BF, tag=f"tmp_in{p}", bufs=1,
                            name=f"tmp_in{p}")
                  for p in range(NP)]
        tmp_out = [dram.tile([OL, OL], BF, tag=f"tmp_out{p}", bufs=1,
                             name=f"tmp_out{p}")
                   for p in range(NP)]

        # ---- tile dicts + load helpers ----
        xn_tiles = {}

        def xn_load(ncn, eng):
            for g in range(NG):
                t = sb.tile([128, 4096], BF, tag="xn", bufs=6,
                            name=f"xn{ncn}_{g}")
                eng.dma_start(t[:], xn_ext[(ncn * NG + g) * 128:
                                           (ncn * NG + g + 1) * 128, :])
                xn_tiles[(ncn, g)] = t

        xt_tiles = {}

        def xt_tile(p, g):
            t = sb.tile([128, 4096], BF, tag="xt", bufs=8, name=f"xt{p}_{g}")
            xt_tiles[(p, g)] = t
            return t

        def load_cols(eng, dst, src_rows, nq):
            """Load [128, 4096] in nq column chunks (subtile deps let
            matmuls start on the first chunk)."""
            w = 4096 // nq
            for q in range(nq):
                eng.dma_start(dst[:, q * w:(q + 1) * w],
                              src_rows[:, q * w:(q + 1) * w])

        # ---- startup: panel-0 xt + psiT emitted in consumption order,
        # alternating the two HWDGE rings per tile ----
        psi_sb = [sb.tile([128, 4096], BF, tag="psi", bufs=NG, name=f"psi{g}")
                  for g in range(NG)]
        for g in range(NG):
            xt_eng, psi_eng = (nc.sync, nc.scalar) if g % 2 == 0 else \
                              (nc.scalar, nc.sync)
            nq = 4 if g == 0 else 2
            load_cols(xt_eng, xt_tile(0, g),
                      xt_ext[(0 * NG + g) * 128:(0 * NG + g + 1) * 128, :], nq)
            load_cols(psi_eng, psi_sb[g],
                      psit_ext[g * 128:(g + 1) * 128, :], nq)

        def xt_load(p):
            for g in range(NG):
                eng = nc.sync if g % 2 == 0 else nc.scalar
                row = (p * NG + g) * 128
                eng.dma_start(xt_tile(p, g)[:], xt_ext[row:row + 128, :])

        xt_load(1)
        # xn chunk 0: needed right at the mm1->mm2 edge; loads during mm1
        xn_load(0, nc.scalar)
        for p in range(NP):
            if p + 2 < NP:
                xt_load(p + 2)
            for dtl in range(4):
                pt = ps.tile([128, 512], F32, tag="ps", bufs=8,
                             name=f"mm1_{p}_{dtl}")
                for g in range(NG):
                    xt = xt_tiles[(p, g)]
                    for kk in range(8):
                        k = 8 * g + kk
                        nc.tensor.matmul(
                            pt[:],
                            xt[:, kk * 512 + dtl * 128:
                               kk * 512 + (dtl + 1) * 128],
                            psi_sb[g][:, kk * 512:(kk + 1) * 512],
                            start=(k == 0), stop=(k == 31))
                st = sb.tile([128, 512], BF, tag="st", bufs=4,
                             name=f"st{p}_{dtl}")
                nc.vector.tensor_copy(st[:], pt[:])
                nc.scalar.dma_start(
                    tmp_in[p][dtl * 128:(dtl + 1) * 128, :], st[:])
            nc.gpsimd.collective_compute(
                "AllReduce", mybir.AluOpType.add, replica_groups=groups,
                ins=[tmp_in[p].opt()], outs=[tmp_out[p].opt()])

        # ---- mm2 stationary tiles (tmpT summed) on sync after xt ----
        ts_tiles = []
        for p in range(NP):
            for j in range(4):
                t = sb.tile([128, 512], BF, tag="ts", bufs=32,
                            name=f"ts{p}_{j}")
                nc.sync.dma_start(t[:], tmp_out[p][j * 128:(j + 1) * 128, :])
                ts_tiles.append(t)

        # ---- remaining xn loads on gpsimd (after all AR triggers) ----
        for ncn in range(1, NCN):
            xn_load(ncn, nc.gpsimd)

        # ---- mm2: out[o, n] = tmpT^T @ x ----
        for ncn in range(NCN):
            mm = [ps.tile([128, 512], F32, tag="ps", bufs=8,
                          name=f"mm2_{ncn}_{ot}")
                  for ot in range(4)]
            for g in range(NG):
                xnt = xn_tiles[(ncn, g)]
                for kk in range(8):
                    kd = 8 * g + kk
                    for ot in range(4):
                        nc.tensor.matmul(
                            mm[ot][:],
                            ts_tiles[kd][:, ot * 128:(ot + 1) * 128],
                            xnt[:, kk * 512:(kk + 1) * 512],
                            start=(kd == 0), stop=(kd == 31))
            for ot in range(4):
                os_ = sb.tile([128, 512], F32, tag="os", bufs=8,
                              name=f"os{ncn}_{ot}")
                nc.vector.tensor_copy(os_[:], mm[ot][:])
                eng = nc.scalar if ot % 2 == 0 else nc.sync
                eng.dma_start(
                    out_ext[ot * 128:(ot + 1) * 128,
                            ncn * 512:(ncn + 1) * 512], os_[:])
    nc.compile()
    return nc


# ---------------- host-side sharding / tiling ----------------
import numpy as np
import ml_dtypes

BF_NP = ml_dtypes.bfloat16

D_FULL, N_FULL, O_FULL = 4096, 8192, 2048
N_CORES = 8


def _tile_k_major(a_bf):
    """[4096 rows, C cols] -> blocks of [128, 8*C'] with k-tiles grouped 8
    per block: in[(8g+kk)*128 + r, c] -> out[(b, r, kk*C512 + c)] per 512-col
    chunk. Works for both xT (chunk axis = d panels) and x natural (chunk
    axis = n chunks).
    Input must be [4096, 4096]. Output [32*128, 4096]."""
    A5 = a_bf.reshape(4, 8, 128, 8, 512)         # g, kk, r, chunk, c
    B = A5.transpose(3, 0, 2, 1, 4)              # chunk, g, r, kk, c
    return np.ascontiguousarray(B).reshape(4096, 4096)


def _tile_psit(psit_bf):
    """[4096, 512] -> [512, 4096]: block g = [128 r, 8 kk * 512 oc]."""
    P4 = psit_bf.reshape(4, 8, 128, 512)         # g, kk, r, oc
    Q = P4.transpose(0, 2, 1, 3)                 # g, r, kk, oc
    return np.ascontiguousarray(Q).reshape(512, 4096)


def make_in_maps(x, Psi, n_cores=8):
    psi_c = (Psi.astype(np.float64)
             - Psi.astype(np.float64).mean(axis=1, keepdims=True))
    psi_c = psi_c.astype(np.float32)
    in_maps = []
    for c in range(n_cores):
        i, j = c % 2, c // 2
        xs = x[:, i * NL:(i + 1) * NL].astype(BF_NP)          # [D, NL]
        xT = np.ascontiguousarray(xs.T)                        # [NL, D]
        ps_ = psi_c[j * OL:(j + 1) * OL, i * NL:(i + 1) * NL].astype(BF_NP)
        psT = np.ascontiguousarray(ps_.T)                      # [NL, OL]
        in_maps.append({
            "xt": _tile_k_major(xT),
            "xn": _tile_k_major(xs),
            "psit": _tile_psit(psT),
        })
    return in_maps


# ---------------- harness-facing wrapper ----------------
_NC_CACHE = {}


def _get_nc():
    if "nc" not in _NC_CACHE:
        _NC_CACHE["nc"] = build_srp_kernel(n_cores=N_CORES, groups=GROUPS)
    return _NC_CACHE["nc"]


def kernel(x, Psi):
    """out = (Psi - rowmean(Psi)) @ x.T @ x on 8 TRN2 NeuronCores."""
    from concourse.bass_utils import run_bass_kernel_spmd
    x = np.asarray(x, dtype=np.float32)
    Psi = np.asarray(Psi, dtype=np.float32)
    assert x.shape == (D_FULL, N_FULL) and Psi.shape == (O_FULL, N_FULL)
    nc = _get_nc()
    in_maps = make_in_maps(x, Psi, n_cores=N_CORES)
    res = run_bass_kernel_spmd(nc, in_maps, core_ids=list(range(N_CORES)))
    out = np.empty((O_FULL, N_FULL), dtype=np.float32)
    for c in range(N_CORES):
        i, j = c % 2, c // 2
        out[j * OL:(j + 1) * OL, i * NL:(i + 1) * NL] = res.results[c]["out"]
    return out
